# revision 12
# baseline (speedup 1.0000x reference)
"""DiGCN inception-block GNN on 8 TRN2 NeuronCores.

Strategy: shard nodes (and their incoming edges) across 8 cores. Per layer:
  x_next = x@lnW + lnb + A1@(x@c1W) + c1b + A2@(x@c2W) + c2b
Since the 128x128 weights commute past the segment-sum, each core collects
bf16 x rows for its edges, scatter-sums them into per-128-node blocks with a
weighted-one-hot matmul accumulated in PSUM (G^T = sum M^T@O), then applies
the three weight matrices per block in a single PSUM accumulation group.
Node features are exchanged between layers with an AllGather.

Key layout choices:
- The weighted one-hot tiles depend only on the (layer-invariant) graph, so
  they are built ONCE on the host and streamed from DRAM each layer as big
  contiguous DMAs — no on-chip one-hot construction at all.
- Layer 0's x is a kernel input, so its per-edge messages are pre-expanded
  on the host too: layer 0 does zero dma_gather work, only contiguous loads.
- Layers 1-2 gather rows from the AllGathered xfull with SWDGE dma_gather
  (int16 indices, SBUF-resident across layers, 4 queues).
- l=0 transposed x tiles are host-provided (no on-chip transposes there).
"""

import sys

sys.path.insert(0, "/opt/trn_rl_repo")

import numpy as np
import ml_dtypes

import concourse.mybir as mybir
import concourse.tile as tile
from concourse import bacc
from concourse import bass_utils

# problem constants (hardcoded per the harness contract)
N = 50000
E = 500000
F = 128
L = 3
NC = 8
P = 128
SH = N // NC          # 6250 nodes per core
BLK = 49              # node blocks per core (49*128 = 6272)
SHP = BLK * P         # 6272 padded shard rows
NFULL = NC * SHP      # 50176 padded full rows
HALF = NFULL // 2     # 25088 (< 32768 so int16 indices work per half)
CALL_T = 8            # tiles per dma_gather call (8*128 = 1024 idx max for
                      # single_packet=True)
OH_T = 32             # tiles per one-hot stream chunk (plain DMA)

BF16 = ml_dtypes.bfloat16


def _pad_row(node):
    return (node // SH) * SHP + (node % SH)


def _prep_edge_set(src, dst, w):
    """Partition one edge set by destination core/block, split by source half."""
    src = np.asarray(src).astype(np.int64)
    dst = np.asarray(dst).astype(np.int64)
    w = np.asarray(w).astype(np.float32)

    core = dst // SH
    blk = (dst % SH) // P
    dloc = (dst % SH) % P
    prow = _pad_row(src)
    half = (prow >= HALF).astype(np.int64)
    idx16 = (prow - half * HALF).astype(np.int64)

    key = (core * BLK + blk) * 2 + half
    order = np.argsort(key, kind="stable")
    key_s = key[order]
    counts = np.bincount(key, minlength=NC * BLK * 2).reshape(NC, BLK, 2)
    T = np.maximum(1, -(-counts // P)).max(axis=0)  # [BLK, 2]

    tile_off = np.zeros((BLK, 2), np.int64)
    tot = [0, 0]
    for h in (0, 1):
        for b in range(BLK):
            tile_off[b, h] = tot[h]
            tot[h] += T[b, h]
    nslots = [int(tot[0]) * P, int(tot[1]) * P]

    starts = np.concatenate([[0], np.cumsum(np.bincount(key_s, minlength=NC * BLK * 2))])
    per_core = []
    for c in range(NC):
        idx_sl = [np.zeros(nslots[h], np.int64) for h in (0, 1)]
        dst_sl = [np.zeros(nslots[h], np.float32) for h in (0, 1)]
        w_sl = [np.zeros(nslots[h], np.float32) for h in (0, 1)]
        for h in (0, 1):
            for b in range(BLK):
                k = (c * BLK + b) * 2 + h
                lo, hi = starts[k], starts[k + 1]
                n = hi - lo
                s0 = tile_off[b, h] * P
                sel = order[lo:hi]
                idx_sl[h][s0:s0 + n] = idx16[sel]
                dst_sl[h][s0:s0 + n] = dloc[sel]
                w_sl[h][s0:s0 + n] = w[sel]
        per_core.append({"idx": idx_sl, "dstf": dst_sl, "wf": w_sl})
    return T, tile_off, per_core


def _wrap_idx(idx_slots):
    n = idx_slots.shape[0]
    a = idx_slots.astype(np.int16).reshape(n // 16, 16).T
    return np.tile(a, (8, 1)).copy()


def _tile_major(slots_2d):
    """[ntiles*128, 128] -> [128, ntiles*128] with tile-major free dim."""
    nt = slots_2d.shape[0] // P
    return np.ascontiguousarray(
        slots_2d.reshape(nt, P, slots_2d.shape[1]).transpose(1, 0, 2)
        .reshape(P, nt * slots_2d.shape[1]))


class _Plan:
    """Compile-time structure shared by all cores (derived from global data)."""

    def __init__(self, T1, off1, T2, off2):
        self.T = [T1, T2]          # [set][BLK, 2] tiles per (block, half)
        self.off = [off1, off2]    # tile offset within (set, half) stream
        self.tot = [[int(T[:, h].sum()) for h in (0, 1)] for T in self.T]
        # chunk/call boundaries: per (set, half) a list of (tile_start, count)
        def chunk(sz):
            calls = [[[] for _ in (0, 1)] for _ in (0, 1)]
            cot = [[{} for _ in (0, 1)] for _ in (0, 1)]
            for s in (0, 1):
                for h in (0, 1):
                    c = 0
                    while c < self.tot[s][h]:
                        n = min(sz, self.tot[s][h] - c)
                        ci = len(calls[s][h])
                        calls[s][h].append((c, n))
                        for t in range(c, c + n):
                            cot[s][h][t] = (ci, t - c)
                        c += n
            return calls, cot
        self.calls, self.call_of_tile = chunk(CALL_T)
        self.ohcalls, self.oh_of_tile = chunk(OH_T)
        self.ncols = [self.tot[s][0] + self.tot[s][1] for s in (0, 1)]


def _build_nc(plan: _Plan, n_layers=L, use_collective=True):
    nc = bacc.Bacc("TRN2", target_bir_lowering=False, debug=False,
                   enable_asserts=False, num_devices=NC, num_swdge_queues=4)
    dt = mybir.dt

    xb_t = nc.dram_tensor("xb", [NFULL, F], dt.bfloat16, kind="ExternalInput")
    xt0_t = nc.dram_tensor("xt0", [P, SHP], dt.bfloat16, kind="ExternalInput")
    wall_t = nc.dram_tensor("wall", [P, 9 * F], dt.bfloat16, kind="ExternalInput")
    bsum_t = nc.dram_tensor("bsum", [P, L], dt.float32, kind="ExternalInput")
    ident_t = nc.dram_tensor("ident", [P, P], dt.bfloat16, kind="ExternalInput")
    iota_t = nc.dram_tensor("iota", [P, P], dt.bfloat16, kind="ExternalInput")
    dst_ts = [nc.dram_tensor(f"dst{s}", [P, plan.ncols[s]], dt.float32,
                             kind="ExternalInput") for s in (0, 1)]
    w_ts = [nc.dram_tensor(f"w{s}", [P, plan.ncols[s]], dt.float32,
                           kind="ExternalInput") for s in (0, 1)]
    idx_ts = [[nc.dram_tensor(f"idx{s}{h}", [P, plan.tot[s][h] * 8], dt.int16,
                              kind="ExternalInput") for h in (0, 1)] for s in (0, 1)]
    # host-precomputed weighted one-hot tiles (tile-major) and layer-0 msgs
    oh_ts = [[nc.dram_tensor(f"oh{s}{h}", [P, plan.tot[s][h] * P], dt.bfloat16,
                             kind="ExternalInput") for h in (0, 1)] for s in (0, 1)]
    m0_ts = [[nc.dram_tensor(f"m0{s}{h}", [P, plan.tot[s][h] * F], dt.bfloat16,
                             kind="ExternalInput") for h in (0, 1)] for s in (0, 1)]
    out_t = nc.dram_tensor("outT", [P, SHP], dt.bfloat16, kind="ExternalOutput")

    with tile.TileContext(nc) as tc:
        with tc.tile_pool(name="const", bufs=1) as constp, \
             tc.tile_pool(name="msg", bufs=6) as msgp, \
             tc.tile_pool(name="ohp", bufs=2) as ohp, \
             tc.tile_pool(name="stage", bufs=4) as stagep, \
             tc.tile_pool(name="gps", bufs=4, space="PSUM") as gpsp, \
             tc.tile_pool(name="ops", bufs=2, space="PSUM") as opsp, \
             tc.tile_pool(name="tps", bufs=2, space="PSUM") as tpsp, \
             tc.tile_pool(name="dram", bufs=2, space="DRAM") as dramp:

            ident_sb = constp.tile([P, P], dt.bfloat16)
            nc.sync.dma_start(out=ident_sb[:], in_=ident_t.ap())
            iota_sb = constp.tile([P, P], dt.bfloat16)
            nc.sync.dma_start(out=iota_sb[:], in_=iota_t.ap())
            dst_sb = [constp.tile([P, plan.ncols[s]], dt.float32,
                                  name=f"dstr{s}") for s in (0, 1)]
            w_sb = [constp.tile([P, plan.ncols[s]], dt.float32,
                                name=f"wr{s}") for s in (0, 1)]
            for s in (0, 1):
                nc.sync.dma_start(out=dst_sb[s][:], in_=dst_ts[s].ap())
                nc.sync.dma_start(out=w_sb[s][:], in_=w_ts[s].ap())
            wall_sb = constp.tile([P, 9 * F], dt.bfloat16)
            nc.sync.dma_start(out=wall_sb[:], in_=wall_t.ap())
            bsum_sb = constp.tile([P, L], dt.float32)
            nc.sync.dma_start(out=bsum_sb[:], in_=bsum_t.ap())

            # layer-resident transposed x: l=0 from host, l=1/2 written
            # on-chip (cycled: layer 2 reuses buffer 0 after layer 0 reads)
            xt_sb = [constp.tile([P, SHP], dt.bfloat16, name=f"xt_l{i}")
                     for i in range(2)]
            nc.sync.dma_start(out=xt_sb[0][:], in_=xt0_t.ap())

            # SBUF-resident gather indices (layers 1-2), loaded once
            idx_sb = [[constp.tile([P, plan.tot[s][h] * 8], dt.int16,
                                   name=f"idxr{s}{h}") for h in (0, 1)]
                      for s in (0, 1)]
            for s in (0, 1):
                for h in (0, 1):
                    nc.sync.dma_start(out=idx_sb[s][h][:], in_=idx_ts[s][h].ap())

            def wsl(l, k):  # lhsT slice for layer l, kind k (0=ln,1=c1,2=c2)
                c0 = (l * 3 + k) * F
                return wall_sb[:, c0:c0 + F]

            agin = [None, None]
            xfull = [None, None]
            for l in range(2):
                agin[l] = dramp.tile([SHP, F], dt.bfloat16, name=f"agin{l}")
                xfull[l] = dramp.tile([NFULL, F], dt.bfloat16,
                                      addr_space="Shared", name=f"xfull{l}")

            qcount = [0]

            for l in range(n_layers):
                src_ap = None if l == 0 else xfull[l - 1][:]
                src_half = (None if l == 0 else
                            [src_ap[0:HALF, :], src_ap[HALF:NFULL, :]])
                lcalls = plan.calls
                lcot = plan.call_of_tile
                call_msg = [[{} for _ in (0, 1)] for _ in (0, 1)]
                call_oh = [[{} for _ in (0, 1)] for _ in (0, 1)]
                emitted = [[0, 0], [0, 0]]
                oh_emitted = [[0, 0], [0, 0]]

                def emit_oh(s, h):
                    ci = oh_emitted[s][h]
                    t0, tcnt = plan.ohcalls[s][h][ci]
                    oh = ohp.tile([P, OH_T, P], dt.bfloat16,
                                  tag=f"oh{s}{h}", name=f"oh_{l}_{s}_{h}_{ci}")
                    if ci % 2 == 0:
                        nc.sync.dma_start(
                            out=oh[:, :tcnt, :],
                            in_=oh_ts[s][h].ap()[:, t0 * P:(t0 + tcnt) * P])
                    else:
                        c0 = 0 if h == 0 else plan.tot[s][0]
                        for t in range(t0, t0 + tcnt):
                            nc.vector.tensor_scalar(
                                out=oh[:, t - t0, :], in0=iota_sb[:],
                                scalar1=dst_sb[s][:, c0 + t:c0 + t + 1],
                                scalar2=w_sb[s][:, c0 + t:c0 + t + 1],
                                op0=mybir.AluOpType.is_equal,
                                op1=mybir.AluOpType.mult)
                    call_oh[s][h][ci] = oh
                    oh_emitted[s][h] += 1

                def emit_call(s, h):
                    ci = emitted[s][h]
                    t0, tcnt = lcalls[s][h][ci]
                    m = msgp.tile([P, CALL_T, F], dt.bfloat16,
                                  tag=f"msg{s}{h}",
                                  name=f"msg_{l}_{s}_{h}_{ci}")
                    if l == 0:
                        nc.sync.dma_start(
                            out=m[:, :tcnt, :],
                            in_=m0_ts[s][h].ap()[:, t0 * F:(t0 + tcnt) * F])
                    else:
                        nc.gpsimd.dma_gather(
                            out_ap=m[:, :tcnt, :],
                            in_ap=src_half[h],
                            idxs_ap=idx_sb[s][h][:, t0 * 8:(t0 + tcnt) * 8],
                            num_idxs=tcnt * P,
                            num_idxs_reg=tcnt * P,
                            elem_size=F,
                            single_packet=True,
                            queue_num=qcount[0] % 4,
                        )
                        qcount[0] += 1
                    call_msg[s][h][ci] = m
                    emitted[s][h] += 1

                def ensure_tiles(s, b):
                    for h in (0, 1):
                        tlast = int(plan.off[s][b, h] + plan.T[s][b, h]) - 1
                        ci_need = lcot[s][h][tlast][0]
                        while emitted[s][h] <= ci_need:
                            emit_call(s, h)
                        oci_need = plan.oh_of_tile[s][h][tlast][0]
                        while oh_emitted[s][h] <= oci_need:
                            emit_oh(s, h)

                for b in range(BLK):
                    ensure_tiles(0, b)
                    ensure_tiles(1, b)

                    # interleaved scatter accumulation chains for both sets
                    gp = []
                    chains = []
                    for s in (0, 1):
                        gp.append(gpsp.tile([P, P], dt.float32, tag="gp",
                                            name=f"gp_{l}_{b}_{s}"))
                        tl = []
                        for h in (0, 1):
                            tb0 = int(plan.off[s][b, h])
                            for t in range(tb0, tb0 + int(plan.T[s][b, h])):
                                tl.append((h, t))
                        chains.append(tl)
                    nmax = max(len(chains[0]), len(chains[1]))
                    for k in range(nmax):
                        for s in (0, 1):
                            if k >= len(chains[s]):
                                continue
                            h, t = chains[s][k]
                            ci, lt = lcot[s][h][t]
                            oci, olt = plan.oh_of_tile[s][h][t]
                            nc.tensor.matmul(
                                out=gp[s][:],
                                lhsT=call_msg[s][h][ci][:, lt, :],
                                rhs=call_oh[s][h][oci][:, olt, :],
                                start=(k == 0),
                                stop=(k == len(chains[s]) - 1),
                            )
                    gs = []
                    for s in (0, 1):
                        gsb = stagep.tile([P, P], dt.bfloat16, tag="gs",
                                          name=f"gs_{l}_{b}_{s}")
                        nc.scalar.copy(out=gsb[:], in_=gp[s][:])
                        gs.append(gsb)

                    xt_b = xt_sb[l % 2][:, b * P:(b + 1) * P]
                    outp = opsp.tile([P, P], dt.float32, tag="outp",
                                     name=f"outp_{l}_{b}")
                    nc.tensor.matmul(out=outp[:], lhsT=wsl(l, 0), rhs=xt_b,
                                     start=True, stop=False)
                    nc.tensor.matmul(out=outp[:], lhsT=wsl(l, 1), rhs=gs[0][:],
                                     start=False, stop=False)
                    nc.tensor.matmul(out=outp[:], lhsT=wsl(l, 2), rhs=gs[1][:],
                                     start=False, stop=True)

                    if l < 2:
                        xt_nb = xt_sb[(l + 1) % 2][:, b * P:(b + 1) * P]
                        nc.vector.tensor_scalar(
                            out=xt_nb, in0=outp[:],
                            scalar1=bsum_sb[:, l:l + 1], scalar2=None,
                            op0=mybir.AluOpType.add)
                        tp2 = tpsp.tile([P, P], dt.bfloat16, tag="tp",
                                        name=f"tp_{l}_{b}")
                        nc.tensor.transpose(out=tp2[:], in_=xt_nb,
                                            identity=ident_sb[:])
                        rm = stagep.tile([P, P], dt.bfloat16, tag="rm",
                                         name=f"rm_{l}_{b}")
                        nc.scalar.copy(out=rm[:], in_=tp2[:])
                        nc.sync.dma_start(
                            out=agin[l][b * P:(b + 1) * P, :], in_=rm[:])
                    else:
                        o32 = stagep.tile([P, P], dt.bfloat16, tag="o32",
                                          name=f"o32_{b}")
                        nc.vector.tensor_scalar(
                            out=o32[:], in0=outp[:],
                            scalar1=bsum_sb[:, 2:3], scalar2=None,
                            op0=mybir.AluOpType.add)
                        nc.sync.dma_start(
                            out=out_t.ap()[:, b * P:(b + 1) * P], in_=o32[:])

                if l < 2 and use_collective:
                    nc.gpsimd.collective_compute(
                        "AllGather",
                        mybir.AluOpType.bypass,
                        replica_groups=[list(range(NC))],
                        ins=[agin[l][:].opt()],
                        outs=[xfull[l][:].opt()],
                    )

    nc.compile()
    return nc


def _host_prep(x, edge_attr, edge_attr2, lnW, lnb, c1W, c1b, c2W, c2b,
               edge_index, edge_index2):
    x = np.asarray(x, np.float32)
    T1, off1, pc1 = _prep_edge_set(edge_index[0], edge_index[1], edge_attr)
    T2, off2, pc2 = _prep_edge_set(edge_index2[0], edge_index2[1], edge_attr2)
    plan = _Plan(T1, off1, T2, off2)

    xb = np.zeros((NFULL, F), BF16)
    xv = x.astype(BF16)
    for c in range(NC):
        xb[c * SHP:c * SHP + SH] = xv[c * SH:(c + 1) * SH]

    wall = np.zeros((P, 9 * F), BF16)
    for l in range(L):
        for k, W in enumerate((lnW, c1W, c2W)):
            wall[:, (l * 3 + k) * F:(l * 3 + k + 1) * F] = \
                np.asarray(W[l], np.float32).astype(BF16)
    bsum = np.stack([
        np.asarray(lnb[l], np.float32) + np.asarray(c1b[l], np.float32)
        + np.asarray(c2b[l], np.float32) for l in range(L)], axis=1)
    ident = np.eye(P, dtype=BF16)
    iota = np.tile(np.arange(P, dtype=BF16), (P, 1))

    in_maps = []
    for c in range(NC):
        m = {
            "xb": xb,
            "xt0": np.ascontiguousarray(xb[c * SHP:(c + 1) * SHP].T),
            "wall": wall,
            "bsum": np.ascontiguousarray(bsum, np.float32),
            "ident": ident,
            "iota": iota,
        }
        for s, pc in ((0, pc1), (1, pc2)):
            dstc = np.zeros((P, plan.ncols[s]), np.float32)
            wc = np.zeros((P, plan.ncols[s]), np.float32)
            for h in (0, 1):
                hb = 0 if h == 0 else plan.tot[s][0]
                nt = plan.tot[s][h]
                dstc[:, hb:hb + nt] = pc[c]["dstf"][h].reshape(nt, P).T
                wc[:, hb:hb + nt] = pc[c]["wf"][h].reshape(nt, P).T
            m[f"dst{s}"] = dstc
            m[f"w{s}"] = wc
            for h in (0, 1):
                nslots = pc[c]["idx"][h].shape[0]
                m[f"idx{s}{h}"] = _wrap_idx(pc[c]["idx"][h])
                # weighted one-hot tiles, tile-major
                ohf = np.zeros((nslots, P), np.float32)
                ohf[np.arange(nslots), pc[c]["dstf"][h].astype(np.int64)] = \
                    pc[c]["wf"][h]
                m[f"oh{s}{h}"] = _tile_major(ohf).astype(BF16)
                # layer-0 pre-expanded messages (raw x rows in slot order)
                idx = pc[c]["idx"][h].astype(np.int64) + h * HALF
                m[f"m0{s}{h}"] = _tile_major(xb[idx])
        in_maps.append(m)
    return plan, in_maps


_CACHE = {}


def _get_compiled(plan_key, plan):
    if plan_key not in _CACHE:
        _CACHE[plan_key] = _build_nc(plan)
    return _CACHE[plan_key]


def kernel(x, edge_attr, edge_attr2, lnW, lnb, c1W, c1b, c2W, c2b,
           edge_index, edge_index2, batch):
    plan, in_maps = _host_prep(x, edge_attr, edge_attr2, lnW, lnb, c1W, c1b,
                               c2W, c2b, edge_index, edge_index2)
    key = (tuple(plan.T[0].ravel()), tuple(plan.T[1].ravel()))
    nc = _get_compiled(key, plan)
    res = bass_utils.run_bass_kernel_spmd(nc, in_maps, core_ids=list(range(NC)))
    out = np.empty((N, F), np.float32)
    for c in range(NC):
        out[c * SH:(c + 1) * SH] = \
            res.results[c]["outT"].T[:SH].astype(np.float32)
    return out


# revision 15
# speedup vs baseline: 1.0315x; 1.0315x over previous
"""DiGCN inception-block GNN on 8 TRN2 NeuronCores.

Strategy: shard nodes (and their incoming edges) across 8 cores. Per layer:
  x_next = x@lnW + lnb + A1@(x@c1W) + c1b + A2@(x@c2W) + c2b
Since the 128x128 weights commute past the segment-sum, each core collects
bf16 x rows for its edges, scatter-sums them into per-128-node blocks with a
weighted-one-hot matmul accumulated in PSUM (G^T = sum M^T@O), then applies
the three weight matrices per block in a single PSUM accumulation group.
Node features are exchanged between layers with an AllGather.

Key layout choices:
- The weighted one-hot tiles depend only on the (layer-invariant) graph, so
  they are built ONCE on the host and streamed from DRAM each layer as big
  contiguous DMAs — no on-chip one-hot construction at all.
- Layer 0's x is a kernel input, so its per-edge messages are pre-expanded
  on the host too: layer 0 does zero dma_gather work, only contiguous loads.
- Layers 1-2 gather rows from the AllGathered xfull with SWDGE dma_gather
  (int16 indices, SBUF-resident across layers, 4 queues).
- l=0 transposed x tiles are host-provided (no on-chip transposes there).
"""

import sys

sys.path.insert(0, "/opt/trn_rl_repo")

import numpy as np
import ml_dtypes

import concourse.mybir as mybir
import concourse.tile as tile
from concourse import bacc
from concourse import bass_utils

# problem constants (hardcoded per the harness contract)
N = 50000
E = 500000
F = 128
L = 3
NC = 8
P = 128
SH = N // NC          # 6250 nodes per core
BLK = 49              # node blocks per core (49*128 = 6272)
SHP = BLK * P         # 6272 padded shard rows
NFULL = NC * SHP      # 50176 padded full rows
ABLK = 25             # blocks in AllGather chunk A (rest in chunk B)
AR = NC * ABLK * P    # 25600 rows in chunk-A tensor (< 32768: int16 ok)
BR = NC * (BLK - ABLK) * P  # 24576 rows in chunk-B tensor
CALL_T = 8            # tiles per dma_gather call (8*128 = 1024 idx max for
                      # single_packet=True)
OH_T = 32             # tiles per one-hot stream chunk (plain DMA)

BF16 = ml_dtypes.bfloat16


def _pad_row(node):
    return (node // SH) * SHP + (node % SH)


def _prep_edge_set(src, dst, w):
    """Partition one edge set by destination core/block, split by source half."""
    src = np.asarray(src).astype(np.int64)
    dst = np.asarray(dst).astype(np.int64)
    w = np.asarray(w).astype(np.float32)

    core = dst // SH
    blk = (dst % SH) // P
    dloc = (dst % SH) % P
    sc = src // SH
    sr = src % SH
    half = (sr // P >= ABLK).astype(np.int64)
    idx16 = np.where(half == 0, sc * (ABLK * P) + sr,
                     sc * ((BLK - ABLK) * P) + (sr - ABLK * P))

    key = (core * BLK + blk) * 2 + half
    order = np.argsort(key, kind="stable")
    key_s = key[order]
    counts = np.bincount(key, minlength=NC * BLK * 2).reshape(NC, BLK, 2)
    T = np.maximum(1, -(-counts // P)).max(axis=0)  # [BLK, 2]

    tile_off = np.zeros((BLK, 2), np.int64)
    tot = [0, 0]
    for h in (0, 1):
        for b in range(BLK):
            tile_off[b, h] = tot[h]
            tot[h] += T[b, h]
    nslots = [int(tot[0]) * P, int(tot[1]) * P]

    starts = np.concatenate([[0], np.cumsum(np.bincount(key_s, minlength=NC * BLK * 2))])
    per_core = []
    for c in range(NC):
        idx_sl = [np.zeros(nslots[h], np.int64) for h in (0, 1)]
        dst_sl = [np.zeros(nslots[h], np.float32) for h in (0, 1)]
        w_sl = [np.zeros(nslots[h], np.float32) for h in (0, 1)]
        for h in (0, 1):
            for b in range(BLK):
                k = (c * BLK + b) * 2 + h
                lo, hi = starts[k], starts[k + 1]
                n = hi - lo
                s0 = tile_off[b, h] * P
                sel = order[lo:hi]
                idx_sl[h][s0:s0 + n] = idx16[sel]
                dst_sl[h][s0:s0 + n] = dloc[sel]
                w_sl[h][s0:s0 + n] = w[sel]
        per_core.append({"idx": idx_sl, "dstf": dst_sl, "wf": w_sl})
    return T, tile_off, per_core


def _wrap_idx(idx_slots):
    n = idx_slots.shape[0]
    a = idx_slots.astype(np.int16).reshape(n // 16, 16).T
    return np.tile(a, (8, 1)).copy()


def _tile_major(slots_2d):
    """[ntiles*128, 128] -> [128, ntiles*128] with tile-major free dim."""
    nt = slots_2d.shape[0] // P
    return np.ascontiguousarray(
        slots_2d.reshape(nt, P, slots_2d.shape[1]).transpose(1, 0, 2)
        .reshape(P, nt * slots_2d.shape[1]))


class _Plan:
    """Compile-time structure shared by all cores (derived from global data)."""

    def __init__(self, T1, off1, T2, off2):
        self.T = [T1, T2]          # [set][BLK, 2] tiles per (block, half)
        self.off = [off1, off2]    # tile offset within (set, half) stream
        self.tot = [[int(T[:, h].sum()) for h in (0, 1)] for T in self.T]
        # chunk/call boundaries: per (set, half) a list of (tile_start, count)
        def chunk(sz):
            calls = [[[] for _ in (0, 1)] for _ in (0, 1)]
            cot = [[{} for _ in (0, 1)] for _ in (0, 1)]
            for s in (0, 1):
                for h in (0, 1):
                    c = 0
                    while c < self.tot[s][h]:
                        n = min(sz, self.tot[s][h] - c)
                        ci = len(calls[s][h])
                        calls[s][h].append((c, n))
                        for t in range(c, c + n):
                            cot[s][h][t] = (ci, t - c)
                        c += n
            return calls, cot
        self.calls, self.call_of_tile = chunk(CALL_T)
        self.ohcalls, self.oh_of_tile = chunk(OH_T)


def _build_nc(plan: _Plan, n_layers=L, use_collective=True):
    nc = bacc.Bacc("TRN2", target_bir_lowering=False, debug=False,
                   enable_asserts=False, num_devices=NC, num_swdge_queues=4)
    dt = mybir.dt

    xt0_t = nc.dram_tensor("xt0", [P, SHP], dt.bfloat16, kind="ExternalInput")
    wall_t = nc.dram_tensor("wall", [P, 9 * F], dt.bfloat16, kind="ExternalInput")
    bsum_t = nc.dram_tensor("bsum", [P, L], dt.float32, kind="ExternalInput")
    ident_t = nc.dram_tensor("ident", [P, P], dt.bfloat16, kind="ExternalInput")
    idx_ts = [[nc.dram_tensor(f"idx{s}{h}", [P, plan.tot[s][h] * 8], dt.int16,
                              kind="ExternalInput") for h in (0, 1)] for s in (0, 1)]
    # host-precomputed weighted one-hot tiles (tile-major) and layer-0 msgs
    oh_ts = [[nc.dram_tensor(f"oh{s}{h}", [P, plan.tot[s][h] * P], dt.bfloat16,
                             kind="ExternalInput") for h in (0, 1)] for s in (0, 1)]
    oh0_ts = [[nc.dram_tensor(f"oh0{s}{h}", [P, plan.tot[s][h] * P], dt.float8e4,
                              kind="ExternalInput") for h in (0, 1)] for s in (0, 1)]
    m0_ts = [[nc.dram_tensor(f"m0{s}{h}", [P, plan.tot[s][h] * F], dt.bfloat16,
                             kind="ExternalInput") for h in (0, 1)] for s in (0, 1)]
    out_t = nc.dram_tensor("outT", [P, SHP], dt.bfloat16, kind="ExternalOutput")

    with tile.TileContext(nc) as tc:
        with tc.tile_pool(name="const", bufs=1) as constp, \
             tc.tile_pool(name="msg", bufs=6) as msgp, \
             tc.tile_pool(name="ohp", bufs=2) as ohp, \
             tc.tile_pool(name="stage", bufs=4) as stagep, \
             tc.tile_pool(name="gps", bufs=4, space="PSUM") as gpsp, \
             tc.tile_pool(name="ops", bufs=2, space="PSUM") as opsp, \
             tc.tile_pool(name="tps", bufs=2, space="PSUM") as tpsp, \
             tc.tile_pool(name="dram", bufs=2, space="DRAM") as dramp:

            ident_sb = constp.tile([P, P], dt.bfloat16)
            nc.sync.dma_start(out=ident_sb[:], in_=ident_t.ap())
            wall_sb = constp.tile([P, 9 * F], dt.bfloat16)
            nc.sync.dma_start(out=wall_sb[:], in_=wall_t.ap())
            bsum_sb = constp.tile([P, L], dt.float32)
            nc.sync.dma_start(out=bsum_sb[:], in_=bsum_t.ap())

            # layer-resident transposed x: l=0 from host, l=1/2 written
            # on-chip (cycled: layer 2 reuses buffer 0 after layer 0 reads)
            xt_sb = [constp.tile([P, SHP], dt.bfloat16, name=f"xt_l{i}")
                     for i in range(2)]
            nc.sync.dma_start(out=xt_sb[0][:], in_=xt0_t.ap())

            # SBUF-resident gather indices (layers 1-2), loaded once
            idx_sb = [[constp.tile([P, plan.tot[s][h] * 8], dt.int16,
                                   name=f"idxr{s}{h}") for h in (0, 1)]
                      for s in (0, 1)]
            for s in (0, 1):
                for h in (0, 1):
                    nc.sync.dma_start(out=idx_sb[s][h][:], in_=idx_ts[s][h].ap())

            def wsl(l, k):  # lhsT slice for layer l, kind k (0=ln,1=c1,2=c2)
                c0 = (l * 3 + k) * F
                return wall_sb[:, c0:c0 + F]

            agin = [None, None]
            xfA = [None, None]
            xfB = [None, None]
            for l in range(2):
                agin[l] = dramp.tile([SHP, F], dt.bfloat16, name=f"agin{l}")
                xfA[l] = dramp.tile([NC, ABLK * P, F], dt.bfloat16,
                                    addr_space="Shared", name=f"xfA{l}")
                xfB[l] = dramp.tile([NC, (BLK - ABLK) * P, F], dt.bfloat16,
                                    addr_space="Shared", name=f"xfB{l}")

            qcount = [0]

            for l in range(n_layers):
                src_half = (None if l == 0 else
                            [xfA[l - 1][:].flatten_outer_dims(),
                             xfB[l - 1][:].flatten_outer_dims()])
                lcalls = plan.calls
                lcot = plan.call_of_tile
                call_msg = [[{} for _ in (0, 1)] for _ in (0, 1)]
                call_oh = [[{} for _ in (0, 1)] for _ in (0, 1)]
                emitted = [[0, 0], [0, 0]]
                oh_emitted = [[0, 0], [0, 0]]

                def emit_oh(s, h):
                    ci = oh_emitted[s][h]
                    t0, tcnt = plan.ohcalls[s][h][ci]
                    odt = dt.float8e4 if l == 0 else dt.bfloat16
                    osrc = oh0_ts if l == 0 else oh_ts
                    oh = ohp.tile([P, OH_T, P], odt,
                                  tag=f"oh{s}{h}", name=f"oh_{l}_{s}_{h}_{ci}")
                    nc.sync.dma_start(
                        out=oh[:, :tcnt, :],
                        in_=osrc[s][h].ap()[:, t0 * P:(t0 + tcnt) * P])
                    call_oh[s][h][ci] = oh
                    oh_emitted[s][h] += 1

                def emit_call(s, h):
                    ci = emitted[s][h]
                    t0, tcnt = lcalls[s][h][ci]
                    m = msgp.tile([P, CALL_T, F], dt.bfloat16,
                                  tag=f"msg{s}{h}",
                                  name=f"msg_{l}_{s}_{h}_{ci}")
                    if l == 0:
                        nc.sync.dma_start(
                            out=m[:, :tcnt, :],
                            in_=m0_ts[s][h].ap()[:, t0 * F:(t0 + tcnt) * F])
                    else:
                        nc.gpsimd.dma_gather(
                            out_ap=m[:, :tcnt, :],
                            in_ap=src_half[h],
                            idxs_ap=idx_sb[s][h][:, t0 * 8:(t0 + tcnt) * 8],
                            num_idxs=tcnt * P,
                            num_idxs_reg=tcnt * P,
                            elem_size=F,
                            single_packet=True,
                            queue_num=qcount[0] % 4,
                        )
                        qcount[0] += 1
                    call_msg[s][h][ci] = m
                    emitted[s][h] += 1

                def ensure_tiles(s, b):
                    for h in (0, 1):
                        tlast = int(plan.off[s][b, h] + plan.T[s][b, h]) - 1
                        ci_need = lcot[s][h][tlast][0]
                        while emitted[s][h] <= ci_need:
                            emit_call(s, h)
                        oci_need = plan.oh_of_tile[s][h][tlast][0]
                        while oh_emitted[s][h] <= oci_need:
                            emit_oh(s, h)

                for b in range(BLK):
                    ensure_tiles(0, b)
                    ensure_tiles(1, b)

                    # interleaved scatter accumulation chains for both sets
                    gp = []
                    chains = []
                    for s in (0, 1):
                        gp.append(gpsp.tile([P, P], dt.float32, tag="gp",
                                            name=f"gp_{l}_{b}_{s}"))
                        tl = []
                        for h in (0, 1):
                            tb0 = int(plan.off[s][b, h])
                            for t in range(tb0, tb0 + int(plan.T[s][b, h])):
                                tl.append((h, t))
                        chains.append(tl)
                    nmax = max(len(chains[0]), len(chains[1]))
                    for k in range(nmax):
                        for s in (0, 1):
                            if k >= len(chains[s]):
                                continue
                            h, t = chains[s][k]
                            ci, lt = lcot[s][h][t]
                            oci, olt = plan.oh_of_tile[s][h][t]
                            nc.tensor.matmul(
                                out=gp[s][:],
                                lhsT=call_msg[s][h][ci][:, lt, :],
                                rhs=call_oh[s][h][oci][:, olt, :],
                                start=(k == 0),
                                stop=(k == len(chains[s]) - 1),
                            )
                    gs = []
                    for s in (0, 1):
                        gsb = stagep.tile([P, P], dt.bfloat16, tag="gs",
                                          name=f"gs_{l}_{b}_{s}")
                        nc.scalar.copy(out=gsb[:], in_=gp[s][:])
                        gs.append(gsb)

                    xt_b = xt_sb[l % 2][:, b * P:(b + 1) * P]
                    outp = opsp.tile([P, P], dt.float32, tag="outp",
                                     name=f"outp_{l}_{b}")
                    nc.tensor.matmul(out=outp[:], lhsT=wsl(l, 0), rhs=xt_b,
                                     start=True, stop=False)
                    nc.tensor.matmul(out=outp[:], lhsT=wsl(l, 1), rhs=gs[0][:],
                                     start=False, stop=False)
                    nc.tensor.matmul(out=outp[:], lhsT=wsl(l, 2), rhs=gs[1][:],
                                     start=False, stop=True)

                    if l < 2:
                        xt_nb = xt_sb[(l + 1) % 2][:, b * P:(b + 1) * P]
                        nc.vector.tensor_scalar(
                            out=xt_nb, in0=outp[:],
                            scalar1=bsum_sb[:, l:l + 1], scalar2=None,
                            op0=mybir.AluOpType.add)
                        tp2 = tpsp.tile([P, P], dt.bfloat16, tag="tp",
                                        name=f"tp_{l}_{b}")
                        nc.tensor.transpose(out=tp2[:], in_=xt_nb,
                                            identity=ident_sb[:])
                        rm = stagep.tile([P, P], dt.bfloat16, tag="rm",
                                         name=f"rm_{l}_{b}")
                        nc.scalar.copy(out=rm[:], in_=tp2[:])
                        nc.sync.dma_start(
                            out=agin[l][b * P:(b + 1) * P, :], in_=rm[:])
                        if b == ABLK - 1 and use_collective:
                            nc.gpsimd.collective_compute(
                                "AllGather",
                                mybir.AluOpType.bypass,
                                replica_groups=[list(range(NC))],
                                ins=[agin[l][0:ABLK * P, :].opt()],
                                outs=[xfA[l][:].opt()],
                            )
                    else:
                        o32 = stagep.tile([P, P], dt.bfloat16, tag="o32",
                                          name=f"o32_{b}")
                        nc.vector.tensor_scalar(
                            out=o32[:], in0=outp[:],
                            scalar1=bsum_sb[:, 2:3], scalar2=None,
                            op0=mybir.AluOpType.add)
                        nc.sync.dma_start(
                            out=out_t.ap()[:, b * P:(b + 1) * P], in_=o32[:])

                if l < 2 and use_collective:
                    nc.gpsimd.collective_compute(
                        "AllGather",
                        mybir.AluOpType.bypass,
                        replica_groups=[list(range(NC))],
                        ins=[agin[l][ABLK * P:SHP, :].opt()],
                        outs=[xfB[l][:].opt()],
                    )

    nc.compile()
    return nc


def _host_prep(x, edge_attr, edge_attr2, lnW, lnb, c1W, c1b, c2W, c2b,
               edge_index, edge_index2):
    x = np.asarray(x, np.float32)
    T1, off1, pc1 = _prep_edge_set(edge_index[0], edge_index[1], edge_attr)
    T2, off2, pc2 = _prep_edge_set(edge_index2[0], edge_index2[1], edge_attr2)
    plan = _Plan(T1, off1, T2, off2)

    xb = np.zeros((NFULL, F), BF16)
    xv = x.astype(BF16)
    for c in range(NC):
        xb[c * SHP:c * SHP + SH] = xv[c * SH:(c + 1) * SH]

    wall = np.zeros((P, 9 * F), BF16)
    for l in range(L):
        for k, W in enumerate((lnW, c1W, c2W)):
            wall[:, (l * 3 + k) * F:(l * 3 + k + 1) * F] = \
                np.asarray(W[l], np.float32).astype(BF16)
    bsum = np.stack([
        np.asarray(lnb[l], np.float32) + np.asarray(c1b[l], np.float32)
        + np.asarray(c2b[l], np.float32) for l in range(L)], axis=1)
    ident = np.eye(P, dtype=BF16)

    in_maps = []
    for c in range(NC):
        m = {
            "xt0": np.ascontiguousarray(xb[c * SHP:(c + 1) * SHP].T),
            "wall": wall,
            "bsum": np.ascontiguousarray(bsum, np.float32),
            "ident": ident,
        }
        for s, pc in ((0, pc1), (1, pc2)):
            for h in (0, 1):
                nslots = pc[c]["idx"][h].shape[0]
                m[f"idx{s}{h}"] = _wrap_idx(pc[c]["idx"][h])
                # weighted one-hot tiles, tile-major
                ohf = np.zeros((nslots, P), np.float32)
                ohf[np.arange(nslots), pc[c]["dstf"][h].astype(np.int64)] = \
                    pc[c]["wf"][h]
                m[f"oh{s}{h}"] = _tile_major(ohf).astype(BF16)
                m[f"oh0{s}{h}"] = _tile_major(
                    (ohf != 0).astype(np.float32)).astype(
                        mybir.dt.np(mybir.dt.float8e4))
                # layer-0 pre-expanded messages, pre-weighted (w folded in,
                # so the layer-0 one-hot is exact 0/1)
                i16 = pc[c]["idx"][h].astype(np.int64)
                if h == 0:
                    gi = (i16 // (ABLK * P)) * SHP + i16 % (ABLK * P)
                else:
                    gi = (i16 // ((BLK - ABLK) * P)) * SHP + ABLK * P \
                        + i16 % ((BLK - ABLK) * P)
                m0 = xb[gi].astype(np.float32) * \
                    pc[c]["wf"][h][:, None]
                m[f"m0{s}{h}"] = _tile_major(m0.astype(BF16))
        in_maps.append(m)
    return plan, in_maps


_CACHE = {}


def _get_compiled(plan_key, plan):
    if plan_key not in _CACHE:
        _CACHE[plan_key] = _build_nc(plan)
    return _CACHE[plan_key]


def kernel(x, edge_attr, edge_attr2, lnW, lnb, c1W, c1b, c2W, c2b,
           edge_index, edge_index2, batch):
    plan, in_maps = _host_prep(x, edge_attr, edge_attr2, lnW, lnb, c1W, c1b,
                               c2W, c2b, edge_index, edge_index2)
    key = (tuple(plan.T[0].ravel()), tuple(plan.T[1].ravel()))
    nc = _get_compiled(key, plan)
    res = bass_utils.run_bass_kernel_spmd(nc, in_maps, core_ids=list(range(NC)))
    out = np.empty((N, F), np.float32)
    for c in range(NC):
        out[c * SH:(c + 1) * SH] = \
            res.results[c]["outT"].T[:SH].astype(np.float32)
    return out
